# revision 6
# baseline (speedup 1.0000x reference)
"""Circular-convolution helper kernel for Trainium2 (8 NeuronCores).

Math: out[i] = sum_b sum_t x1[b,(i-t)%D] * x2[b,t]
            = sum_j G[j, (i-j)%D]   where G = x1^T @ x2  ([D, D], K=B contraction)

Sharding: G's rows are split across the 8 cores (core c owns rows
[128c, 128c+128)).  Each core:
  1. A = x1c^T @ x2  (single K=128 matmul into PSUM, column-chunked)
  2. coarse circular shift per 16-partition group via GPSIMD indirect_copy:
       H1[p, i] = A[p, (i - 16*(p//16)) % D]
  3. collapse the 8 groups with a selection matmul (sel16[p, f] = [p%16 == f]):
       C[f, i] = sum_g H1[16g+f, i] = sum_g A[16g+f, (i-16g) % D]
  4. ship C [16, D] to the host.
Host: out = sum_c roll_{128c}( sum_f roll_f(C_c[f]) ) — the remaining fine
shifts (f < 16) and the cross-core combine are O(cores*16*D) host work.
"""

import numpy as np

B = 128
DIM = 1024
NCORES = 8
CHUNK = DIM // NCORES  # 128 rows of G per core
NHALF = 512  # PSUM bank = 512 fp32
NCHUNKS = 4  # column chunks for pipelining
CW = DIM // NCHUNKS  # 256


_cached = {}


def _build():
    if "nc" in _cached:
        return _cached["nc"]

    import concourse.bass as bass
    import concourse.mybir as mybir
    from concourse import bacc
    from concourse.tile import TileContext

    f32 = mybir.dt.float32
    u16 = mybir.dt.uint16

    nc = bacc.Bacc("TRN2", target_bir_lowering=False, debug=False)

    x1c = nc.dram_tensor("x1c", [B, CHUNK], f32, kind="ExternalInput")
    x2 = nc.dram_tensor("x2", [B, DIM], f32, kind="ExternalInput")
    out = nc.dram_tensor("out", [16, DIM], f32, kind="ExternalOutput")

    # constant selection matrix: sel16[p, f] = 1.0 iff p % 16 == f
    sel_np = np.tile(np.eye(16, dtype=np.float32), (8, 1))
    sel_dram = nc.inline_tensor(sel_np, name="sel16")

    # constant gather indices for the per-group coarse shift:
    # logical index vector of group g is v_g[i] = (i - 16g) % D, stored
    # wrapped across the group's 16 partitions: idx[16g + (i%16), i//16]
    p_arr = np.arange(B)
    s_arr = np.arange(DIM // 16)
    idx_np = (
        (16 * s_arr[None, :] + 2 * (p_arr[:, None] & 15) - p_arr[:, None]) % DIM
    ).astype(np.uint16)
    idx_dram = nc.inline_tensor(idx_np, name="gidx")

    with TileContext(nc) as tc:
        with (
            tc.tile_pool(name="sb", bufs=1) as sb,
            tc.tile_pool(name="ps", bufs=1, space="PSUM") as ps,
        ):
            # ---- input loads (x2 split across 2 DMAs for queue parallelism)
            x1t = sb.tile([B, CHUNK], f32)
            nc.sync.dma_start(x1t[:], x1c.ap())
            x2t = sb.tile([B, DIM], f32)
            x2ap = x2.ap()
            for h in range(2):
                nc.sync.dma_start(
                    x2t[:, h * NHALF : (h + 1) * NHALF],
                    x2ap[:, h * NHALF : (h + 1) * NHALF],
                )
            selt = sb.tile([B, 16], f32)
            nc.sync.dma_start(selt[:], sel_dram.ap())
            idx = sb.tile([B, DIM // 16], u16)
            nc.sync.dma_start(idx[:], idx_dram.ap())

            # ---- G matmul (column-chunked), PSUM->SBUF copy, gather, collapse
            g = ps.tile([CHUNK, DIM], f32)
            a = sb.tile([CHUNK, DIM], f32)
            h1 = sb.tile([CHUNK, DIM], f32)
            c_ps = ps.tile([16, DIM], f32)

            for ch in range(NCHUNKS):
                lo, hi = ch * CW, (ch + 1) * CW
                nc.tensor.matmul(
                    g[:, lo:hi], x1t[:], x2t[:, lo:hi], start=True, stop=True
                )
                # alternate copy engine to halve the copy span
                eng = nc.scalar if ch % 2 == 0 else nc.vector
                if ch % 2 == 0:
                    nc.scalar.copy(a[:, lo:hi], g[:, lo:hi])
                else:
                    nc.vector.tensor_copy(a[:, lo:hi], g[:, lo:hi])

            # gather works on full columns (wraparound): single op on Pool
            nc.gpsimd.indirect_copy(h1[:], a[:], idx[:], True)

            for ch in range(NCHUNKS):
                lo, hi = ch * CW, (ch + 1) * CW
                nc.tensor.matmul(
                    c_ps[:, lo:hi], selt[:], h1[:, lo:hi], start=True, stop=True
                )

            cs = sb.tile([16, DIM], f32)
            nc.scalar.copy(cs[:, 0:NHALF], c_ps[:, 0:NHALF])
            nc.vector.tensor_copy(cs[:, NHALF:DIM], c_ps[:, NHALF:DIM])
            nc.sync.dma_start(out.ap(), cs[:])

    nc.compile()
    _cached["nc"] = nc
    return nc


def _in_maps(input1, input2):
    x1 = np.ascontiguousarray(np.asarray(input1, dtype=np.float32))
    x2 = np.ascontiguousarray(np.asarray(input2, dtype=np.float32))
    return [
        {
            "x1c": np.ascontiguousarray(x1[:, c * CHUNK : (c + 1) * CHUNK]),
            "x2": x2,
        }
        for c in range(NCORES)
    ]


def _combine(results):
    total = np.zeros(DIM, np.float64)
    rolls = np.arange(DIM)
    for c in range(NCORES):
        C = results[c]["out"].astype(np.float64)  # [16, DIM]
        part = np.zeros(DIM, np.float64)
        for f in range(16):
            part += np.roll(C[f], f)
        total += np.roll(part, CHUNK * c)
    return total.astype(np.float32).reshape(1, 1, DIM)


def _run(input1, input2, **kwargs):
    from concourse import bass_utils

    nc = _build()
    res = bass_utils.run_bass_kernel_spmd(
        nc, _in_maps(input1, input2), core_ids=list(range(NCORES)), **kwargs
    )
    return res


def kernel(input1, input2):
    res = _run(input1, input2)
    return _combine(res.results)


# revision 8
# speedup vs baseline: 1.7690x; 1.7690x over previous
"""Circular-convolution helper kernel for Trainium2 (8 NeuronCores).

Math: out[i] = sum_b sum_t x1[b,(i-t)%D] * x2[b,t]
            = sum_j G[j, (i-j)%D]   where G = x1^T @ x2  ([D, D], K=B contraction)

Sharding: G's rows are split across the 8 cores (core c owns rows
[128c, 128c+128)).  Per core, column-chunked and pipelined:
  1. A = x1c^T @ x2 into PSUM (fp32 K=128 matmul, 4 column chunks)
  2. PSUM -> SBUF copy (Scalar/Vector alternate)
  3. scatter A into a DRAM buffer gd with row pitch 2D so that the circular
     anti-diagonals become rows: gd[m, k] = A[m, k % D] for k in [897, 2048)
     (only the columns the diagonal read touches are written)
  4. diagonal read H[m, i] = gd[m, 1024 + i - m] = A[m, (i-m) % D]
     (row stride 2D-1 in the flat DRAM view)
  5. ones-matmul partition collapse: part[i] = sum_m H[m, i]
Host rotates each core's partial by 128c and sums.

Chunks are processed in order 3,0,1,2 so the wrap-around tail (A columns
[896,1024), needed by the first diagonal-read chunk) lands in DRAM early,
letting reads stream right behind the writes.
"""

import numpy as np

B = 128
DIM = 1024
NCORES = 8
CHUNK = DIM // NCORES  # 128 rows of G per core
NHALF = 512
NCHUNKS = 4
CW = DIM // NCHUNKS  # 256

USE_F32R = False  # single-pass fp32r G-matmul (reduced precision streaming)


_cached = {}


def _build():
    key = ("nc", USE_F32R)
    if key in _cached:
        return _cached[key]

    import concourse.bass as bass
    import concourse.mybir as mybir
    from concourse import bacc
    from concourse.tile import TileContext

    f32 = mybir.dt.float32
    bf16 = mybir.dt.bfloat16

    nc = bacc.Bacc("TRN2", target_bir_lowering=False, debug=False)

    x1c = nc.dram_tensor("x1c", [B, CHUNK], f32, kind="ExternalInput")
    x2 = nc.dram_tensor("x2", [B, DIM], f32, kind="ExternalInput")
    out = nc.dram_tensor("out", [1, DIM], f32, kind="ExternalOutput")
    # diag scratch: row pitch 2D, only cols [896, 2048) ever written/read
    gd = nc.dram_tensor("gd", [CHUNK, 2 * DIM], f32, kind="Internal")

    with TileContext(nc) as tc:
        with (
            tc.tile_pool(name="sb", bufs=1) as sb,
            tc.tile_pool(name="ps", bufs=1, space="PSUM") as ps,
        ):
            order = [3, 0, 1, 2]

            x1t = sb.tile([B, CHUNK], f32)
            nc.sync.dma_start(x1t[:], x1c.ap())
            x2t = sb.tile([B, DIM], f32)
            x2ap = x2.ap()
            for ch in order:
                lo, hi = ch * CW, (ch + 1) * CW
                nc.sync.dma_start(x2t[:, lo:hi], x2ap[:, lo:hi])

            g = ps.tile([CHUNK, DIM], f32)
            a = sb.tile([CHUNK, DIM], f32)
            ht = sb.tile([CHUNK, DIM], f32)
            ones = sb.tile([CHUNK, 1], f32)
            nc.vector.memset(ones[:], 1.0)
            o = ps.tile([1, DIM], f32)
            gd_ap = gd.ap()

            if USE_F32R:
                f32r = mybir.dt.float32r
                x1_mm = x1t[:].bitcast(f32r)
                x2_mm = x2t[:].bitcast(f32r)
            else:
                x1_mm = x1t[:]
                x2_mm = x2t[:]

            for i, ch in enumerate(order):
                lo, hi = ch * CW, (ch + 1) * CW
                # 1. G chunk
                nc.tensor.matmul(
                    g[:, lo:hi], x1_mm, x2_mm[:, lo:hi], start=True, stop=True
                )
                # 2. PSUM -> SBUF with bf16 cast, alternating engines
                if i % 2 == 0:
                    nc.scalar.copy(a[:, lo:hi], g[:, lo:hi])
                else:
                    nc.vector.tensor_copy(a[:, lo:hi], g[:, lo:hi])
                # 3. scatter to gd second copy: gd[:, D+lo : D+hi]
                nc.sync.dma_start(gd_ap[:, DIM + lo : DIM + hi], a[:, lo:hi])
                if ch == 3:
                    # wrap tail: gd[:, 896:1024] = A[:, 896:1024]
                    nc.sync.dma_start(gd_ap[:, 896:DIM], a[:, 896:DIM])

            # 4+5. diagonal reads + bf16 ones-matmul collapse, streamed
            for ch in range(NCHUNKS):
                lo, hi = ch * CW, (ch + 1) * CW
                diag = bass.AP(gd, DIM + lo, [[2 * DIM - 1, CHUNK], [1, CW]])
                nc.sync.dma_start(ht[:, lo:hi], diag)
                nc.tensor.matmul(
                    o[:, lo:hi], ones[:], ht[:, lo:hi], start=True, stop=True
                )

            ot = sb.tile([1, DIM], f32)
            nc.scalar.copy(ot[:, 0:NHALF], o[:, 0:NHALF])
            nc.vector.tensor_copy(ot[:, NHALF:DIM], o[:, NHALF:DIM])
            nc.sync.dma_start(out.ap(), ot[:])

    nc.compile()
    _cached[key] = nc
    return nc


def _in_maps(input1, input2):
    x1 = np.ascontiguousarray(np.asarray(input1, dtype=np.float32))
    x2 = np.ascontiguousarray(np.asarray(input2, dtype=np.float32))
    return [
        {
            "x1c": np.ascontiguousarray(x1[:, c * CHUNK : (c + 1) * CHUNK]),
            "x2": x2,
        }
        for c in range(NCORES)
    ]


def _combine(results):
    total = np.zeros(DIM, np.float64)
    for c in range(NCORES):
        total += np.roll(results[c]["out"][0].astype(np.float64), CHUNK * c)
    return total.astype(np.float32).reshape(1, 1, DIM)


def _run(input1, input2, **kwargs):
    from concourse import bass_utils

    nc = _build()
    res = bass_utils.run_bass_kernel_spmd(
        nc, _in_maps(input1, input2), core_ids=list(range(NCORES)), **kwargs
    )
    return res


def kernel(input1, input2):
    res = _run(input1, input2)
    return _combine(res.results)
